# revision 3
# baseline (speedup 1.0000x reference)
"""Trainium2 Bass kernel for nn_DPSpikingDecoder.

Math: the leaky-integrator scan v_t = 0.5*v_{t-1} + x_t, the mean over
channels C, and the differential window pooling are all linear maps over
the time axis, and the scan kernel is identical for every channel.  So

    dp[b, w, f] = sum_{c,t} (K[w, t] / C) * spikes[b, c, t, f]

where K = M_pool @ L_scan is a [W=40, T=960] matrix precomputed on host.
Viewing spikes[b] as a flat [C*T, F] matrix, this is one 30720-long
matmul contraction per sample, streamed through the PE in 240 chunks of
128 rows while spikes stream from HBM exactly once (memory-bound; the
per-core HBM roofline is ~358 GB/s).  The weight tile for chunk m
depends only on m mod 15 (lcm(128, 960) = 1920 = 15*128), so 15 weight
tiles stay resident in SBUF.  float32r gives the full-rate PE path
(1 cycle/row at N=256) on unmodified fp32 bytes.

HBM layout: the host pre-transposes each sample into partition-major
tiles x[d][p][s][f] = flat[3072 d + 128 s + p, f], so every DMA reads
128 fully-contiguous 24 KB partition lines (one descriptor each) instead
of strided 1 KB elements -- this is the difference between ~22 GB/s and
~27 GiB/s per SDMA engine.  Nine 3 MB tiles alternate between the two
HWDGE rings; the last tile is read as four 768 KB slices so the final
matmuls drain as soon as each lands.  W1 rides mid-stream so the MLP
tail never waits on it.

The tiny MLP + softmax + scale run on-chip as a short tail; layer 1 is
packed 4-wide into PE column groups via tile_position.

Sharding: data-parallel over batch B=8 -> one sample per NeuronCore.
"""

import numpy as np
from contextlib import ExitStack

import concourse.bass as bass
import concourse.bacc as bacc
import concourse.tile as tile
from concourse import mybir
from concourse.bass_utils import run_bass_kernel_spmd

F32 = mybir.dt.float32
F32R = mybir.dt.float32r

B, C, T, F = 8, 32, 960, 256
L_DP, N_DP = 24, 12
W = T // L_DP            # 40 windows
H = 20                   # hidden dim of the MLP

R = C * T                # 30720 contraction rows per sample
# 120-row chunks on partitions 0..119: SDMA engines 13/15 (which serve
# partitions 120..127 / 92..95+124..127) are the chronically slow ones;
# partitions 120..127 are never touched, so engine 15 carries a half
# share and never straggles the end-of-stream semaphores, while the
# other 14 engines still exceed the per-core HBM limit.
CH = 120                 # rows per matmul chunk (partitions 0..119)
NCH = R // CH            # 256 chunks
QP = 8                   # weight-tile period: lcm(120, 960) / 120
CPD = 32                 # chunks per streamed tile (3.75 MB, one DMA each)
NT = NCH // CPD          # 8 tiles; the last one is read in 4 slices
FQ = CPD // 4            # chunks per final-tile slice


def _host_K():
    """K[w, t] in float64: differential pooling of the decayed scan."""
    t = np.arange(T)
    d = t[:, None] - t[None, :]
    Lmat = np.where(d >= 0, 0.5 ** np.clip(d, 0, None), 0.0)
    M = np.zeros((W, T))
    for w in range(W):
        M[w, w * L_DP + L_DP - N_DP : w * L_DP + L_DP] = 1.0 / N_DP
        M[w, w * L_DP : w * L_DP + N_DP] -= 1.0 / N_DP
    return M @ Lmat  # [W, T]


def _host_kt():
    """SBUF image [CH, QP*W]: kt[p, q*W+w] = K[w, (128q+p)%960]/C."""
    K = _host_K()
    q = np.arange(QP)[:, None]
    p = np.arange(CH)[None, :]
    tidx = (CH * q + p) % T                      # [QP, CH]
    kt2 = K.T[tidx] / C                          # [QP, CH, W]
    img = kt2.transpose(1, 0, 2).reshape(CH, QP * W)
    return np.ascontiguousarray(img.astype(np.float32))


def _host_cimg(W2, b2):
    """Packed small consts, one contiguous [128, 101] DMA image:
    cols 0:40 eye(40) on parts 0:40; 40:80 [W2; b2] on parts 0:21;
    col 80 b1 placeholder (zeros, real b1 patched in kernel());
    cols 81:101 the 4-col-group summing matrix."""
    img = np.zeros((128, 101), dtype=np.float32)
    img[0:W, 0:W] = np.eye(W, dtype=np.float32)
    img[0:H, 40:80] = W2.astype(np.float32)
    img[H, 40:80] = b2.astype(np.float32)
    for j in range(4):
        for i in range(H):
            img[32 * j + i, 81 + i] = 1.0
    return img


def _build_program():
    nc = bacc.Bacc(None)
    x = nc.declare_dram_parameter("x", [NT, CH, CPD, F], F32R, isOutput=False)
    kt = nc.declare_dram_parameter("kt", [CH, QP * W], F32R, isOutput=False)
    w1r = nc.declare_dram_parameter("w1r", [128, 2 * W * H], F32, isOutput=False)
    cimg = nc.declare_dram_parameter("cimg", [128, 101], F32, isOutput=False)
    y = nc.declare_dram_parameter("y", [W, F], F32, isOutput=True)

    with tile.TileContext(nc) as tc, ExitStack() as ctx:
        consts = ctx.enter_context(tc.tile_pool(name="consts", bufs=1))
        xs = ctx.enter_context(tc.tile_pool(name="xs", bufs=3))
        work = ctx.enter_context(tc.tile_pool(name="work", bufs=1))
        dp_psp = ctx.enter_context(tc.tile_pool(name="dp_ps", bufs=1, space="PSUM"))
        sm_ps = ctx.enter_context(tc.tile_pool(name="sm_ps", bufs=1, space="PSUM"))

        # kt first on the sync ring (the PE needs it for the first MM);
        # cimg on the scalar ring; both are tiny and contiguous.
        kt_sb = consts.tile([CH, QP, W], F32R)
        nc.sync.dma_start(out=kt_sb, in_=kt[:].rearrange("p (q w) -> p q w", q=QP))
        ci_sb = consts.tile([128, 101], F32)
        nc.scalar.dma_start(out=ci_sb, in_=cimg[:])
        eye_sb = ci_sb[0:W, 0:W]
        w2b_sb = ci_sb[0 : H + 1, 40:80]
        b1_sb = ci_sb[0:H, 80:81]
        sel_sb = ci_sb[:, 81:101]
        w1_sb = consts.tile([128, 2 * W * H], F32)

        # augmented MLP input [h; 1] so layer 2 adds b2 inside the matmul
        h_aug = work.tile([H + 1, 1], F32)
        nc.vector.memset(h_aug, 1.0)  # row H stays 1; rows 0..H-1 overwritten

        # ---- big streamed contraction: dp[w, f] += kt_q^T @ x_chunk ----
        # One 3 MB fully-contiguous DMA per tile, alternating rings.
        dp_ps = dp_psp.tile([W, F], F32)
        for d in range(NT - 1):
            xt = xs.tile([CH, CPD, F], F32R)
            eng = nc.sync if d % 2 == 0 else nc.scalar
            eng.dma_start(out=xt, in_=x[d])
            # w1 rides mid-stream behind tile 4/5 so it is resident long
            # before the tail, and the final x slices are not delayed.
            if d == 4:
                nc.sync.dma_start(out=w1_sb[:, 0 : W * H], in_=w1r[:, 0 : W * H])
            if d == 5:
                nc.scalar.dma_start(out=w1_sb[:, W * H :], in_=w1r[:, W * H :])
            for s in range(CPD):
                m = d * CPD + s
                nc.tensor.matmul(
                    dp_ps,
                    lhsT=kt_sb[:, m % QP, :],
                    rhs=xt[:, s, :],
                    start=(m == 0),
                    stop=False,
                )
        # last tile arrives as four quarter-DMAs so the final matmuls can
        # drain as soon as each 6-chunk slice lands
        d = NT - 1
        for qd in range(4):
            xt_q = xs.tile([CH, FQ, F], F32R, tag="xt_q", bufs=4)
            eng = nc.sync if qd % 2 == 0 else nc.scalar
            eng.dma_start(out=xt_q, in_=x[d, :, qd * FQ : (qd + 1) * FQ, :])
            for s2 in range(FQ):
                m = d * CPD + qd * FQ + s2
                nc.tensor.matmul(
                    dp_ps,
                    lhsT=kt_sb[:, m % QP, :],
                    rhs=xt_q[:, s2, :],
                    start=False,
                    stop=(m == NCH - 1),
                )

        dp_sb = work.tile([W, F], F32)
        nc.vector.tensor_copy(dp_sb, dp_ps)

        # ---- transpose dp to feed the MLP contraction ----
        dpT_ps = sm_ps.tile([128, 2, W], F32)
        for e in range(2):
            nc.tensor.transpose(dpT_ps[:, e, :], dp_sb[:, e * 128 : (e + 1) * 128], eye_sb)
        dpT_sb = work.tile([128, 2, W], F32)
        nc.vector.tensor_copy(dpT_sb, dpT_ps)

        # ---- layer 1: h = relu(dp_flat @ W1 + b1), 80 chunks of 128 ----
        # packed 4-wide into PE column groups; partial sums land in four
        # partition slices of hp_ps and are summed by one sel-matmul.
        hp_ps = sm_ps.tile([128, 1], F32)
        for m in range(2 * W):
            w, e = divmod(m, 2)
            j = m % 4
            nc.tensor.matmul(
                hp_ps[32 * j : 32 * j + H, :],
                lhsT=w1_sb[:, m * H : (m + 1) * H],
                rhs=dpT_sb[:, e, w : w + 1],
                start=(m < 4),
                stop=(m >= 2 * W - 4),
                tile_position=(0, 32 * j),
            )
        hp_sb = work.tile([128, 1], F32)
        nc.vector.tensor_copy(hp_sb, hp_ps)
        h_ps = sm_ps.tile([H, 1], F32)
        nc.tensor.matmul(h_ps, lhsT=sel_sb, rhs=hp_sb, start=True, stop=True)
        nc.scalar.activation(
            h_aug[0:H, :], h_ps, mybir.ActivationFunctionType.Relu, bias=b1_sb
        )

        # ---- layer 2 (+b2 via augmented row) + softmax on a [1, W] row ----
        a2_ps = sm_ps.tile([1, W], F32)
        nc.tensor.matmul(a2_ps, lhsT=h_aug, rhs=w2b_sb, start=True, stop=True)
        e_sb = work.tile([1, W], F32)
        ssum = work.tile([1, 1], F32)
        nc.scalar.activation(
            e_sb, a2_ps, mybir.ActivationFunctionType.Exp, accum_out=ssum[:]
        )
        rin = work.tile([1, 1], F32)
        nc.vector.reciprocal(rin, ssum)
        ta_sb = work.tile([1, W], F32)
        nc.vector.tensor_scalar_mul(ta_sb, e_sb, rin[:])

        # ---- scale dp rows by attention weights and store ----
        taT_ps = sm_ps.tile([W, 1], F32)
        nc.tensor.transpose(taT_ps, ta_sb, ci_sb[0:1, 0:1])
        ta_col = work.tile([W, 1], F32)
        nc.vector.tensor_copy(ta_col, taT_ps)
        att = work.tile([W, F], F32)
        for e2 in range(2):
            nc.vector.tensor_scalar_mul(
                att[:, e2 * 128 : (e2 + 1) * 128],
                dp_sb[:, e2 * 128 : (e2 + 1) * 128],
                ta_col[:],
            )
        nc.sync.dma_start(out=y[:], in_=att[:])

    nc.compile()
    return nc


_CACHED = {}


def _get_program():
    if "nc" not in _CACHED:
        _CACHED["nc"] = _build_program()
        _CACHED["kt"] = _host_kt()
    return _CACHED["nc"]


def _in_maps(spikes, W1, b1, W2, b2):
    spikes = np.asarray(spikes, dtype=np.float32)
    W1 = np.asarray(W1, dtype=np.float32)
    b1 = np.asarray(b1, dtype=np.float32)
    W2 = np.asarray(W2, dtype=np.float32)
    b2 = np.asarray(b2, dtype=np.float32)
    _get_program()
    # W1 rearranged so chunk m = 2*w + e holds rows d = 256*w + 128*e + p,
    # laid out so the DMA is one contiguous [128, 1600] block.
    w1r = np.ascontiguousarray(
        W1.reshape(W, 2, 128, H).transpose(2, 0, 1, 3).reshape(128, 2 * W * H)
    )
    cimg = _host_cimg(W2, b2)
    cimg[0:H, 80] = b1
    shared = {"kt": _CACHED["kt"], "w1r": w1r, "cimg": cimg}
    # partition-major tile layout: x[d, p, s, f] = flat[3072 d + 128 s + p, f]
    return [
        {
            "x": np.ascontiguousarray(
                spikes[b].reshape(NT, CPD, CH, F).transpose(0, 2, 1, 3)
            ),
            **shared,
        }
        for b in range(B)
    ]


def kernel(spikes, W1, b1, W2, b2):
    in_maps = _in_maps(spikes, W1, b1, W2, b2)
    res = run_bass_kernel_spmd(_get_program(), in_maps, list(range(B)))
    out = np.stack([np.asarray(res.results[i]["y"]).reshape(W * F) for i in range(B)])
    return out.astype(np.float32)


# revision 4
# speedup vs baseline: 1.7475x; 1.7475x over previous
"""Trainium2 Bass kernel for nn_DPSpikingDecoder.

Math: the leaky-integrator scan v_t = 0.5*v_{t-1} + x_t, the mean over
channels C, and the differential window pooling are all linear maps over
the time axis, and the scan kernel is identical for every channel.  So

    dp[b, w, f] = sum_{c,t} (K[w, t] / C) * spikes[b, c, t, f]

where K = M_pool @ L_scan is a [W=40, T=960] matrix precomputed on host.
Viewing spikes[b] as a flat [C*T, F] matrix, this is one 30720-long
matmul contraction per sample, streamed through the PE while spikes
stream from HBM exactly once (memory-bound; the per-core HBM roofline
is ~358 GB/s).  float32r gives the full-rate PE path on unmodified
fp32 bytes.

HBM layout / DMA shape: the host pre-transposes each sample into
partition-major tiles so every DMA reads fully-contiguous multi-KB
partition lines (one descriptor each).  Each 3.84 MB tile (= 4 channels)
is read as a [128, 27, F] main DMA plus a [64, 6, F] companion DMA on
partitions 0..63.  The companion shifts ~12% of the bytes onto the even
SDMA engines only: SDMA engine 15 (partitions 92-95/124-127) runs ~25%
slower than its peers on this part, and with a plain 1/16 share it
drags the end of the stream by ~10 us while everyone else sits idle.
With the companion, engine 15's share takes the same wall clock as the
even engines' enlarged share, so the stream finishes in one clean edge.
(Partition counts other than 128/64 hit a ~2.5x-slower descriptor path
- measured - so the rebalance must use a 64-partition shape.)

The tiny MLP + softmax + scale run on-chip as a short tail; layer 1 is
packed 4-wide into PE column groups via tile_position.

Sharding: data-parallel over batch B=8 -> one sample per NeuronCore.
"""

import numpy as np
from contextlib import ExitStack

import concourse.bass as bass
import concourse.bacc as bacc
import concourse.tile as tile
from concourse import mybir
from concourse.bass_utils import run_bass_kernel_spmd

F32 = mybir.dt.float32
F32R = mybir.dt.float32r

B, C, T, F = 8, 32, 960, 256
L_DP, N_DP = 24, 12
W = T // L_DP            # 40 windows
H = 20                   # hidden dim of the MLP

R = C * T                # 30720 contraction rows per sample
NT = 8                   # tiles; one tile = 4 channels = 3840 rows
SA = 27                  # full [128, F] chunks per tile (main DMA)
SB = 6                   # half [64, F] chunks per tile (companion DMA)
FQ = 9                   # chunks per final-tile main sub-DMA (3 of them)


def _host_K():
    """K[w, t] in float64: differential pooling of the decayed scan."""
    t = np.arange(T)
    d = t[:, None] - t[None, :]
    Lmat = np.where(d >= 0, 0.5 ** np.clip(d, 0, None), 0.0)
    M = np.zeros((W, T))
    for w in range(W):
        M[w, w * L_DP + L_DP - N_DP : w * L_DP + L_DP] = 1.0 / N_DP
        M[w, w * L_DP : w * L_DP + N_DP] -= 1.0 / N_DP
    return M @ Lmat  # [W, T]


def _host_kt():
    """SBUF image [128, (SA+SB)*W]:
    cols q<SA:   kt[p, q*W+w] = K[w, (128q+p)%960]/C        (main chunks)
    cols q>=SA:  kt[p, q*W+w] = K[w, 576+64(q-SA)+p]/C, p<64 (companion)."""
    K = _host_K()
    img = np.zeros((128, (SA + SB) * W), dtype=np.float64)
    p = np.arange(128)
    for q in range(SA):
        tidx = (128 * q + p) % T
        img[:, q * W : (q + 1) * W] = K.T[tidx] / C
    p64 = np.arange(64)
    for u in range(SB):
        tidx = 576 + 64 * u + p64
        img[0:64, (SA + u) * W : (SA + u + 1) * W] = K.T[tidx] / C
    return np.ascontiguousarray(img.astype(np.float32))


def _host_cimg(W2, b2):
    """Packed small consts, one contiguous [128, 101] DMA image:
    cols 0:40 eye(40) on parts 0:40; 40:80 [W2; b2] on parts 0:21;
    col 80 b1 placeholder (zeros, real b1 patched in kernel());
    cols 81:101 the 4-col-group summing matrix."""
    img = np.zeros((128, 101), dtype=np.float32)
    img[0:W, 0:W] = np.eye(W, dtype=np.float32)
    img[0:H, 40:80] = W2.astype(np.float32)
    img[H, 40:80] = b2.astype(np.float32)
    for j in range(4):
        for i in range(H):
            img[32 * j + i, 81 + i] = 1.0
    return img


def _build_program():
    nc = bacc.Bacc(None)
    xa = nc.declare_dram_parameter("xa", [NT, 128, SA, F], F32R, isOutput=False)
    xb = nc.declare_dram_parameter("xb", [NT, 64, SB, F], F32R, isOutput=False)
    kt = nc.declare_dram_parameter("kt", [128, (SA + SB) * W], F32R, isOutput=False)
    w1r = nc.declare_dram_parameter("w1r", [128, 2 * W * H], F32, isOutput=False)
    cimg = nc.declare_dram_parameter("cimg", [128, 101], F32, isOutput=False)
    y = nc.declare_dram_parameter("y", [W, F], F32, isOutput=True)

    with tile.TileContext(nc) as tc, ExitStack() as ctx:
        consts = ctx.enter_context(tc.tile_pool(name="consts", bufs=1))
        xs = ctx.enter_context(tc.tile_pool(name="xs", bufs=3))
        xbs = ctx.enter_context(tc.tile_pool(name="xbs", bufs=3))
        work = ctx.enter_context(tc.tile_pool(name="work", bufs=1))
        dp_psp = ctx.enter_context(tc.tile_pool(name="dp_ps", bufs=1, space="PSUM"))
        sm_ps = ctx.enter_context(tc.tile_pool(name="sm_ps", bufs=1, space="PSUM"))

        # kt first on the sync ring (the PE needs it for the first MM);
        # cimg on the scalar ring; both are tiny and contiguous.
        kt_sb = consts.tile([128, SA + SB, W], F32R)
        nc.sync.dma_start(out=kt_sb, in_=kt[:].rearrange("p (q w) -> p q w", q=SA + SB))
        ci_sb = consts.tile([128, 101], F32)
        nc.scalar.dma_start(out=ci_sb, in_=cimg[:])
        eye_sb = ci_sb[0:W, 0:W]
        w2b_sb = ci_sb[0 : H + 1, 40:80]
        b1_sb = ci_sb[0:H, 80:81]
        sel_sb = ci_sb[:, 81:101]
        w1_sb = consts.tile([128, 2 * W * H], F32)

        # augmented MLP input [h; 1] so layer 2 adds b2 inside the matmul
        h_aug = work.tile([H + 1, 1], F32)
        nc.vector.memset(h_aug, 1.0)  # row H stays 1; rows 0..H-1 overwritten

        # ---- big streamed contraction: dp[w, f] += kt_q^T @ x_chunk ----
        # Per tile: one [128, 27, F] main DMA + one [64, 6, F] companion
        # on the opposite ring.
        dp_ps = dp_psp.tile([W, F], F32)
        for d in range(NT):
            last = d == NT - 1
            ea = nc.sync if d % 2 == 0 else nc.scalar
            eb = nc.scalar if d % 2 == 0 else nc.sync
            xtb = xbs.tile([64, SB, F], F32R)
            eb.dma_start(out=xtb, in_=xb[d])
            if not last:
                xt = xs.tile([128, SA, F], F32R)
                ea.dma_start(out=xt, in_=xa[d])
                sub = [(xt, 0, SA)]
            else:
                # final tile's main part arrives as three sub-DMAs so the
                # last matmuls drain as soon as each 9-chunk slice lands
                sub = []
                for qd in range(3):
                    xt_q = xs.tile([128, FQ, F], F32R, tag="xt_q", bufs=3)
                    eng = ea if qd % 2 == 0 else eb
                    eng.dma_start(out=xt_q, in_=xa[d, :, qd * FQ : (qd + 1) * FQ, :])
                    sub.append((xt_q, qd * FQ, FQ))
            # w1 rides mid-stream so it is resident long before the tail
            if d == 4:
                nc.sync.dma_start(out=w1_sb[:, 0 : W * H], in_=w1r[:, 0 : W * H])
                nc.scalar.dma_start(out=w1_sb[:, W * H :], in_=w1r[:, W * H :])
            # companion (64-row) chunks first: their DMA is small and lands
            # quickly on the opposite ring
            for u in range(SB):
                nc.tensor.matmul(
                    dp_ps,
                    lhsT=kt_sb[0:64, SA + u, :],
                    rhs=xtb[:, u, :],
                    start=(d == 0 and u == 0),
                    stop=False,
                )
            for xt_i, s0, ns in sub:
                for s in range(ns):
                    nc.tensor.matmul(
                        dp_ps,
                        lhsT=kt_sb[:, s0 + s, :],
                        rhs=xt_i[:, s, :],
                        start=False,
                        stop=(last and s0 + s == SA - 1),
                    )

        dp_sb = work.tile([W, F], F32)
        nc.vector.tensor_copy(dp_sb, dp_ps)

        # ---- transpose dp to feed the MLP contraction ----
        dpT_ps = sm_ps.tile([128, 2, W], F32)
        for e in range(2):
            nc.tensor.transpose(dpT_ps[:, e, :], dp_sb[:, e * 128 : (e + 1) * 128], eye_sb)
        dpT_sb = work.tile([128, 2, W], F32)
        nc.vector.tensor_copy(dpT_sb, dpT_ps)

        # ---- layer 1: h = relu(dp_flat @ W1 + b1), 80 chunks of 128 ----
        # packed 4-wide into PE column groups; partial sums land in four
        # partition slices of hp_ps and are summed by one sel-matmul.
        hp_ps = sm_ps.tile([128, 1], F32)
        for m in range(2 * W):
            w, e = divmod(m, 2)
            j = m % 4
            nc.tensor.matmul(
                hp_ps[32 * j : 32 * j + H, :],
                lhsT=w1_sb[:, m * H : (m + 1) * H],
                rhs=dpT_sb[:, e, w : w + 1],
                start=(m < 4),
                stop=(m >= 2 * W - 4),
                tile_position=(0, 32 * j),
            )
        hp_sb = work.tile([128, 1], F32)
        nc.vector.tensor_copy(hp_sb, hp_ps)
        h_ps = sm_ps.tile([H, 1], F32)
        nc.tensor.matmul(h_ps, lhsT=sel_sb, rhs=hp_sb, start=True, stop=True)
        nc.scalar.activation(
            h_aug[0:H, :], h_ps, mybir.ActivationFunctionType.Relu, bias=b1_sb
        )

        # ---- layer 2 (+b2 via augmented row) + softmax on a [1, W] row ----
        a2_ps = sm_ps.tile([1, W], F32)
        nc.tensor.matmul(a2_ps, lhsT=h_aug, rhs=w2b_sb, start=True, stop=True)
        e_sb = work.tile([1, W], F32)
        ssum = work.tile([1, 1], F32)
        nc.scalar.activation(
            e_sb, a2_ps, mybir.ActivationFunctionType.Exp, accum_out=ssum[:]
        )
        rin = work.tile([1, 1], F32)
        nc.vector.reciprocal(rin, ssum)
        ta_sb = work.tile([1, W], F32)
        nc.vector.tensor_scalar_mul(ta_sb, e_sb, rin[:])

        # ---- scale dp rows by attention weights and store ----
        taT_ps = sm_ps.tile([W, 1], F32)
        nc.tensor.transpose(taT_ps, ta_sb, ci_sb[0:1, 0:1])
        ta_col = work.tile([W, 1], F32)
        nc.vector.tensor_copy(ta_col, taT_ps)
        att = work.tile([W, F], F32)
        for e2 in range(2):
            nc.vector.tensor_scalar_mul(
                att[:, e2 * 128 : (e2 + 1) * 128],
                dp_sb[:, e2 * 128 : (e2 + 1) * 128],
                ta_col[:],
            )
        nc.sync.dma_start(out=y[:], in_=att[:])

    nc.compile()
    return nc


_CACHED = {}


def _get_program():
    if "nc" not in _CACHED:
        _CACHED["nc"] = _build_program()
        _CACHED["kt"] = _host_kt()
    return _CACHED["nc"]


def _in_maps(spikes, W1, b1, W2, b2):
    spikes = np.asarray(spikes, dtype=np.float32)
    W1 = np.asarray(W1, dtype=np.float32)
    b1 = np.asarray(b1, dtype=np.float32)
    W2 = np.asarray(W2, dtype=np.float32)
    b2 = np.asarray(b2, dtype=np.float32)
    _get_program()
    # W1 rearranged so chunk m = 2*w + e holds rows d = 256*w + 128*e + p,
    # laid out so the DMA is one contiguous [128, 1600] block.
    w1r = np.ascontiguousarray(
        W1.reshape(W, 2, 128, H).transpose(2, 0, 1, 3).reshape(128, 2 * W * H)
    )
    cimg = _host_cimg(W2, b2)
    cimg[0:H, 80] = b1
    shared = {"kt": _CACHED["kt"], "w1r": w1r, "cimg": cimg}
    # partition-major layouts:
    #  xa[d, p, s, f] = flat[3840 d + 128 s + p, f]          (rows 0..3455)
    #  xb[d, p, u, f] = flat[3840 d + 3456 + 64 u + p, f]    (rows 3456..3839)
    maps = []
    for b in range(B):
        flat = spikes[b].reshape(NT, 3840, F)
        xa_ = np.ascontiguousarray(
            flat[:, : SA * 128, :].reshape(NT, SA, 128, F).transpose(0, 2, 1, 3)
        )
        xb_ = np.ascontiguousarray(
            flat[:, SA * 128 :, :].reshape(NT, SB, 64, F).transpose(0, 2, 1, 3)
        )
        maps.append({"xa": xa_, "xb": xb_, **shared})
    return maps


def kernel(spikes, W1, b1, W2, b2):
    in_maps = _in_maps(spikes, W1, b1, W2, b2)
    res = run_bass_kernel_spmd(_get_program(), in_maps, list(range(B)))
    out = np.stack([np.asarray(res.results[i]["y"]).reshape(W * F) for i in range(B)])
    return out.astype(np.float32)


# revision 11
# speedup vs baseline: 1.8154x; 1.0388x over previous
"""Trainium2 Bass kernel for nn_DPSpikingDecoder.

Math: the leaky-integrator scan v_t = 0.5*v_{t-1} + x_t, the mean over
channels C, and the differential window pooling are all linear maps over
the time axis, and the scan kernel is identical for every channel.  So

    dp[b, w, f] = sum_{c,t} (K[w, t] / C) * spikes[b, c, t, f]

where K = M_pool @ L_scan is a [W=40, T=960] matrix precomputed on host.
Viewing spikes[b] as a flat [C*T, F] matrix, this is one 30720-long
matmul contraction per sample, streamed through the PE in 240 chunks of
128 rows while spikes stream from HBM exactly once (memory-bound; the
per-core HBM roofline is ~358 GB/s).  The weight tile for chunk m
depends only on m mod 15 (lcm(128, 960) = 1920 = 15*128), so 15 weight
tiles stay resident in SBUF.  float32r gives the full-rate PE path
(1 cycle/row at N=256) on unmodified fp32 bytes.

HBM layout: the host pre-transposes each sample into partition-major
tiles x[v][p][s][f] = flat[768 v + 128 s + p, f], so every DMA reads
128 fully-contiguous 6 KB partition lines (one descriptor each) instead
of strided 1 KB elements.  6 KB is the measured sweet spot: large
enough to amortize per-descriptor overhead (~22 GB/s/engine at 1 KB),
small enough that SDMA engine 15 -- which degrades to ~21 GB/s on
24 KB lines while its peers run 26 GB/s -- holds line rate and never
builds the ~10 us end-of-stream backlog seen with 3 MB tile DMAs.
40 sub-DMAs alternate between the two HWDGE rings; W1 rides mid-stream
so the MLP tail never waits on it.

The tiny MLP + softmax + scale run on-chip as a short tail; layer 1 is
packed 4-wide into PE column groups via tile_position.

Sharding: data-parallel over batch B=8 -> one sample per NeuronCore.
"""

import numpy as np
from contextlib import ExitStack

import concourse.bass as bass
import concourse.bacc as bacc
import concourse.tile as tile
from concourse import mybir
from concourse.bass_utils import run_bass_kernel_spmd

F32 = mybir.dt.float32
F32R = mybir.dt.float32r

B, C, T, F = 8, 32, 960, 256
L_DP, N_DP = 24, 12
W = T // L_DP            # 40 windows
H = 20                   # hidden dim of the MLP

R = C * T                # 30720 contraction rows per sample
CH = 128                 # rows per matmul chunk
NCH = R // CH            # 240 chunks
QP = 15                  # weight-tile period: lcm(128, 960) / 128
SW = 6                   # chunks per streamed sub-DMA (768 KB, 6 KB lines)
NW = NCH // SW           # 40 sub-DMAs


def _host_K():
    """K[w, t] in float64: differential pooling of the decayed scan."""
    t = np.arange(T)
    d = t[:, None] - t[None, :]
    Lmat = np.where(d >= 0, 0.5 ** np.clip(d, 0, None), 0.0)
    M = np.zeros((W, T))
    for w in range(W):
        M[w, w * L_DP + L_DP - N_DP : w * L_DP + L_DP] = 1.0 / N_DP
        M[w, w * L_DP : w * L_DP + N_DP] -= 1.0 / N_DP
    return M @ Lmat  # [W, T]


def _host_kt():
    """SBUF image [CH, QP*W]: kt[p, q*W+w] = K[w, (128q+p)%960]/C."""
    K = _host_K()
    q = np.arange(QP)[:, None]
    p = np.arange(CH)[None, :]
    tidx = (CH * q + p) % T                      # [QP, CH]
    kt2 = K.T[tidx] / C                          # [QP, CH, W]
    img = kt2.transpose(1, 0, 2).reshape(CH, QP * W)
    return np.ascontiguousarray(img.astype(np.float32))


def _host_cimg(W2, b2):
    """Packed small consts, one contiguous [128, 101] DMA image:
    cols 0:40 eye(40) on parts 0:40; 40:80 [W2; b2] on parts 0:21;
    col 80 b1 placeholder (zeros, real b1 patched in kernel());
    cols 81:101 the 4-col-group summing matrix."""
    img = np.zeros((128, 101), dtype=np.float32)
    img[0:W, 0:W] = np.eye(W, dtype=np.float32)
    img[0:H, 40:80] = W2.astype(np.float32)
    img[H, 40:80] = b2.astype(np.float32)
    for j in range(4):
        for i in range(H):
            img[32 * j + i, 81 + i] = 1.0
    return img


def _build_program():
    nc = bacc.Bacc(None)
    x = nc.declare_dram_parameter("x", [NW, CH, SW, F], F32R, isOutput=False)
    kt = nc.declare_dram_parameter("kt", [CH, QP * W], F32R, isOutput=False)
    w1r = nc.declare_dram_parameter("w1r", [128, 2 * W * H], F32, isOutput=False)
    cimg = nc.declare_dram_parameter("cimg", [128, 101], F32, isOutput=False)
    y = nc.declare_dram_parameter("y", [W, F], F32, isOutput=True)

    with tile.TileContext(nc) as tc, ExitStack() as ctx:
        consts = ctx.enter_context(tc.tile_pool(name="consts", bufs=1))
        xs = ctx.enter_context(tc.tile_pool(name="xs", bufs=8))
        work = ctx.enter_context(tc.tile_pool(name="work", bufs=1))
        dp_psp = ctx.enter_context(tc.tile_pool(name="dp_ps", bufs=1, space="PSUM"))
        sm_ps = ctx.enter_context(tc.tile_pool(name="sm_ps", bufs=1, space="PSUM"))

        # kt first on the sync ring (the PE needs it for the first MM);
        # cimg on the scalar ring; both are tiny and contiguous.
        kt_sb = consts.tile([CH, QP, W], F32R)
        nc.sync.dma_start(out=kt_sb, in_=kt[:].rearrange("p (q w) -> p q w", q=QP))
        ci_sb = consts.tile([128, 101], F32)
        nc.scalar.dma_start(out=ci_sb, in_=cimg[:])
        eye_sb = ci_sb[0:W, 0:W]
        w2b_sb = ci_sb[0 : H + 1, 40:80]
        b1_sb = ci_sb[0:H, 80:81]
        sel_sb = ci_sb[:, 81:101]
        w1_sb = consts.tile([128, 2 * W * H], F32)

        # augmented MLP input [h; 1] so layer 2 adds b2 inside the matmul
        h_aug = work.tile([H + 1, 1], F32)
        nc.vector.memset(h_aug, 1.0)  # row H stays 1; rows 0..H-1 overwritten

        # ---- big streamed contraction: dp[w, f] += kt_q^T @ x_chunk ----
        # 40 sub-DMAs of [128, 6, F] alternating rings.  6 KB partition
        # lines are the sweet spot: big enough to amortize descriptor
        # overhead, small enough that SDMA engine 15 (which degrades to
        # ~21 GB/s on 24 KB lines but runs ~29 GB/s on 6 KB ones) never
        # builds a backlog that drags the end of the stream.  Completions
        # arrive every ~2 us, which also keeps the PE HAM-warm.
        dp_ps = dp_psp.tile([W, F], F32)
        for v in range(NW):
            xt = xs.tile([CH, SW, F], F32R)
            eng = nc.sync if v % 2 == 0 else nc.scalar
            eng.dma_start(out=xt, in_=x[v])
            # w1 rides mid-stream so it is resident long before the tail
            if v == 20:
                nc.sync.dma_start(out=w1_sb[:, 0 : W * H], in_=w1r[:, 0 : W * H])
            if v == 21:
                nc.scalar.dma_start(out=w1_sb[:, W * H :], in_=w1r[:, W * H :])
            for s in range(SW):
                m = v * SW + s
                nc.tensor.matmul(
                    dp_ps,
                    lhsT=kt_sb[:, m % QP, :],
                    rhs=xt[:, s, :],
                    start=(m == 0),
                    stop=(m == NCH - 1),
                )

        dp_sb = work.tile([W, F], F32)
        nc.vector.tensor_copy(dp_sb, dp_ps)

        # ---- transpose dp to feed the MLP contraction ----
        dpT_ps = sm_ps.tile([128, 2, W], F32)
        for e in range(2):
            nc.tensor.transpose(dpT_ps[:, e, :], dp_sb[:, e * 128 : (e + 1) * 128], eye_sb)
        dpT_sb = work.tile([128, 2, W], F32)
        nc.vector.tensor_copy(dpT_sb, dpT_ps)

        # ---- layer 1: h = relu(dp_flat @ W1 + b1), 80 chunks of 128 ----
        # packed 4-wide into PE column groups; partial sums land in four
        # partition slices of hp_ps and are summed by one sel-matmul.
        hp_ps = sm_ps.tile([128, 1], F32)
        for m in range(2 * W):
            w, e = divmod(m, 2)
            j = m % 4
            nc.tensor.matmul(
                hp_ps[32 * j : 32 * j + H, :],
                lhsT=w1_sb[:, m * H : (m + 1) * H],
                rhs=dpT_sb[:, e, w : w + 1],
                start=(m < 4),
                stop=(m >= 2 * W - 4),
                tile_position=(0, 32 * j),
            )
        hp_sb = work.tile([128, 1], F32)
        nc.vector.tensor_copy(hp_sb, hp_ps)
        h_ps = sm_ps.tile([H, 1], F32)
        nc.tensor.matmul(h_ps, lhsT=sel_sb, rhs=hp_sb, start=True, stop=True)
        nc.scalar.activation(
            h_aug[0:H, :], h_ps, mybir.ActivationFunctionType.Relu, bias=b1_sb
        )

        # ---- layer 2 (+b2 via augmented row) + softmax on a [1, W] row ----
        a2_ps = sm_ps.tile([1, W], F32)
        nc.tensor.matmul(a2_ps, lhsT=h_aug, rhs=w2b_sb, start=True, stop=True)
        e_sb = work.tile([1, W], F32)
        ssum = work.tile([1, 1], F32)
        nc.scalar.activation(
            e_sb, a2_ps, mybir.ActivationFunctionType.Exp, accum_out=ssum[:]
        )
        rin = work.tile([1, 1], F32)
        nc.vector.reciprocal(rin, ssum)
        ta_sb = work.tile([1, W], F32)
        nc.vector.tensor_scalar_mul(ta_sb, e_sb, rin[:])

        # ---- scale dp rows by attention weights and store ----
        taT_ps = sm_ps.tile([W, 1], F32)
        nc.tensor.transpose(taT_ps, ta_sb, ci_sb[0:1, 0:1])
        ta_col = work.tile([W, 1], F32)
        nc.vector.tensor_copy(ta_col, taT_ps)
        att = work.tile([W, F], F32)
        for e2 in range(2):
            nc.vector.tensor_scalar_mul(
                att[:, e2 * 128 : (e2 + 1) * 128],
                dp_sb[:, e2 * 128 : (e2 + 1) * 128],
                ta_col[:],
            )
        nc.sync.dma_start(out=y[:], in_=att[:])

    nc.compile()
    return nc


_CACHED = {}


def _get_program():
    if "nc" not in _CACHED:
        _CACHED["nc"] = _build_program()
        _CACHED["kt"] = _host_kt()
    return _CACHED["nc"]


def _in_maps(spikes, W1, b1, W2, b2):
    spikes = np.asarray(spikes, dtype=np.float32)
    W1 = np.asarray(W1, dtype=np.float32)
    b1 = np.asarray(b1, dtype=np.float32)
    W2 = np.asarray(W2, dtype=np.float32)
    b2 = np.asarray(b2, dtype=np.float32)
    _get_program()
    # W1 rearranged so chunk m = 2*w + e holds rows d = 256*w + 128*e + p,
    # laid out so the DMA is one contiguous [128, 1600] block.
    w1r = np.ascontiguousarray(
        W1.reshape(W, 2, 128, H).transpose(2, 0, 1, 3).reshape(128, 2 * W * H)
    )
    cimg = _host_cimg(W2, b2)
    cimg[0:H, 80] = b1
    shared = {"kt": _CACHED["kt"], "w1r": w1r, "cimg": cimg}
    # partition-major tile layout: x[v, p, s, f] = flat[768 v + 128 s + p, f]
    return [
        {
            "x": np.ascontiguousarray(
                spikes[b].reshape(NW, SW, CH, F).transpose(0, 2, 1, 3)
            ),
            **shared,
        }
        for b in range(B)
    ]


def kernel(spikes, W1, b1, W2, b2):
    in_maps = _in_maps(spikes, W1, b1, W2, b2)
    res = run_bass_kernel_spmd(_get_program(), in_maps, list(range(B)))
    out = np.stack([np.asarray(res.results[i]["y"]).reshape(W * F) for i in range(B)])
    return out.astype(np.float32)
